# revision 16
# baseline (speedup 1.0000x reference)
"""Trainium2 Bass kernel for the nn_Decoder LSTM problem — pipelined edition.

Same time-sharded Picard-sweep algorithm as the baseline (8 cores x 1072-step
chunks, fp8 DoubleRow bulk sweeps + fp16 polish, exact DVE scan for the linear
c recurrence), restructured for PE occupancy:

  - jobs = (sweep, tile) pairs; per-tile gate buffers (tags bufs=3/3/2) so the
    next job's gate/psA/ct0 matmuls are emitted into the current job's
    scan-ladder windows (software pipelining across tiles AND sweeps).
  - boundary flush runs directly on the vector engine reading the live gate
    buffers (no gpsimd copy chain, no deferral machinery).
  - recurrent-weight DMA queued after the first x/wx slabs so phase 1 starts
    immediately.
"""

import os
import sys
import numpy as np
import ml_dtypes

for _p in ("/opt/trn_rl_repo", "/root/.axon_site/_ro/trn_rl_repo"):
    if _p not in sys.path:
        sys.path.insert(0, _p)

import concourse.bass as bass
import concourse.bacc as bacc
import concourse.mybir as mybir
import concourse.tile as tile
from concourse.bass_utils import run_bass_kernel_spmd
from contextlib import ExitStack

D = 1024
T = 8192
KC = 8
DELTA = 48
L = 1024 + DELTA
N_CORES = 8
WS = 16.0          # fp8 weight scale
SCHEDULE = os.environ.get("LSTM_SCHED", "z88888ff")
NI = os.environ.get("LSTM_NI", "12")   # n_inner per sweep (last char repeats)

F16 = mybir.dt.float16
F32 = mybir.dt.float32
F8 = mybir.dt.float8e4
AF = mybir.ActivationFunctionType
ALU = mybir.AluOpType
DR = mybir.MatmulPerfMode.DoubleRow

M_TILES = [(0, 512), (512, 512), (1024, L - 1024)]


def build_nc(schedule=SCHEDULE, ni_str=NI):
    def ni_of(si):
        return int(ni_str[min(si, len(ni_str) - 1)])

    nc = bacc.Bacc(None, target_bir_lowering=False, debug=False)

    # ---- I/O ----
    wh8_d = nc.declare_dram_parameter("wh8", [D, 4 * D], F8, isOutput=False)
    wc8_d = nc.declare_dram_parameter("wc8", [D, D], F8, isOutput=False)
    wh16_d = nc.declare_dram_parameter("wh16", [D, 4 * D], F16, isOutput=False)
    wc16_d = nc.declare_dram_parameter("wc16", [D, D], F16, isOutput=False)
    wx_d = nc.declare_dram_parameter("wx", [D, 3 * D], F16, isOutput=False)
    x_d = nc.declare_dram_parameter("x", [D, L], F16, isOutput=False)
    bias_fio = nc.declare_dram_parameter("bias_fio", [128, 3, KC], F32, isOutput=False)
    bc_d = nc.declare_dram_parameter("bc", [128, KC], F32, isOutput=False)
    tanh_bc_d = nc.declare_dram_parameter("tanh_bc", [128, KC], F32, isOutput=False)
    id16_d = nc.declare_dram_parameter("id16", [128, 128], F16, isOutput=False)
    id1_d = nc.declare_dram_parameter("id1", [128, 128], F16, isOutput=False)
    hb16_d = nc.declare_dram_parameter("hb16", [128, KC, 1], F16, isOutput=False)
    cb16_d = nc.declare_dram_parameter("cb16", [128, KC, 1], F16, isOutput=False)
    hb8_d = nc.declare_dram_parameter("hb8", [128, KC, 1], F8, isOutput=False)
    cb8_d = nc.declare_dram_parameter("cb8", [128, KC, 1], F8, isOutput=False)
    h_out = nc.declare_dram_parameter("h_out", [128, KC, L + 1], F16, isOutput=True)

    pre_dram = nc.dram_tensor("pre_scratch", [3 * KC, 128, L], F16)

    # sweep descriptors: schedule[0] must be 'z' (fused with phase 1)
    assert schedule[0] == "z"
    sweeps = [(si, m, ni_of(si)) for si, m in enumerate(schedule)]
    last8_si = max((si for si, m, _ in sweeps if m == "8"), default=None)

    with tile.TileContext(nc) as tc:
        with (
            tc.tile_pool(name="const", bufs=1) as constp,
            tc.tile_pool(name="psum", bufs=4, space="PSUM") as psum,
            tc.tile_pool(name="state", bufs=1) as spool,
            tc.tile_pool(name="gates", bufs=1) as gpool,
            tc.tile_pool(name="work", bufs=1) as wk,
            tc.tile_pool(name="prestream", bufs=2) as prepool,
            tc.tile_pool(name="tiny", bufs=2) as tp,
        ):
            _nmc = [0]

            def _nm(p):
                _nmc[0] += 1
                return f"{p}{_nmc[0]}"

            wstack = ExitStack()
            fstack = ExitStack()
            p16 = {}   # filled with fpool/wcc pool at the 8->f boundary

            bfio_sb = constp.tile([128, 3, KC], F32, tag="bfio")
            bc_sb = constp.tile([128, KC], F32, tag="bc")
            tbc_sb = constp.tile([128, KC], F32, tag="tbc")
            id16_sb = constp.tile([128, 128], F16, tag="id16")
            id1_sb = constp.tile([128, 128], F16, tag="id1")
            nc.sync.dma_start(bfio_sb[...], bias_fio[...])
            nc.sync.dma_start(bc_sb[...], bc_d[...])
            nc.sync.dma_start(tbc_sb[...], tanh_bc_d[...])
            nc.sync.dma_start(id16_sb[...], id16_d[...])
            nc.sync.dma_start(id1_sb[...], id1_d[...])

            wc8_sb = constp.tile([128, KC, D], F8, tag="wc8")
            wh8_ref = {}   # wh8_ref["sb"] set when the wh8 pool opens

            LP = L + 4
            Hf = spool.tile([128, KC, LP], F16, tag="H")
            Cf = spool.tile([128, KC, LP], F16, tag="C")
            H8 = spool.tile([128, KC, LP], F8, tag="H8")
            C8 = spool.tile([128, KC, LP], F8, tag="C8")
            nc.sync.dma_start(Hf[:, :, 0:1], hb16_d[...])
            nc.sync.dma_start(Cf[:, :, 0:1], cb16_d[...])
            nc.sync.dma_start(H8[:, :, 0:1], hb8_d[...])
            nc.sync.dma_start(C8[:, :, 0:1], cb8_d[...])

            # ------------- shared helpers -------------
            def do_scan(t0, N, fgt, bb, ch):
                nc.vector.tensor_tensor_scan(
                    Cf[:, ch, t0 + 1:t0 + N],
                    fgt[:, ch, :N - 1],
                    bb[:, ch, :N - 1],
                    Cf[:, ch, t0:t0 + 1],
                    ALU.mult, ALU.add,
                )

            def gate8(garr, g, t0, N, pre_tiles):
                for cq in range(4):
                    ps = psum.tile([128, 2, 512], F32, tag="pp")
                    for j in range(2):
                        ch = cq * 2 + j
                        col = g * D + ch * 128
                        for kc in range(0, KC, 2):
                            nc.tensor.matmul(
                                ps[:, j, :N],
                                wh8_ref["sb"][:, kc:kc + 2, col:col + 128],
                                H8[:, kc:kc + 2, t0:t0 + N],
                                start=(kc == 0), stop=False, perf_mode=DR)
                        nc.tensor.matmul(
                            ps[:, j, :N], id16_sb[:, :],
                            pre_tiles[ch][:, :N], start=False, stop=True)
                    nc.scalar.activation(
                        garr[:, cq * 2:cq * 2 + 2, :N], ps[:, :, :N],
                        AF.Sigmoid, scale=1.0 / WS)

            def psA8(zA, t0, N):
                for cq in range(4):
                    psA = psum.tile([128, 2, 512], F32, tag="pp")
                    for j in range(2):
                        ch = cq * 2 + j
                        col = 3 * D + ch * 128
                        for kc in range(0, KC, 2):
                            nc.tensor.matmul(
                                psA[:, j, :N],
                                wh8_ref["sb"][:, kc:kc + 2, col:col + 128],
                                H8[:, kc:kc + 2, t0:t0 + N],
                                start=(kc == 0), stop=(kc == KC - 2),
                                perf_mode=DR)
                    for j in range(2):
                        ch = cq * 2 + j
                        nc.scalar.activation(
                            zA[:, ch, :N], psA[:, j, :N], AF.Identity,
                            bias=bc_sb[:, ch:ch + 1], scale=1.0 / WS)

            def ct0_8(ct, zA, t0, N):
                for cq in range(4):
                    ps = psum.tile([128, 2, 512], F32, tag="pp")
                    for j in range(2):
                        ch = cq * 2 + j
                        nc.tensor.matmul(
                            ps[:, j, :N], id16_sb[:, :], zA[:, ch, :N],
                            start=True, stop=False)
                        for kc in range(0, KC, 2):
                            nc.tensor.matmul(
                                ps[:, j, :N],
                                wc8_sb[:, kc:kc + 2, ch * 128:(ch + 1) * 128],
                                C8[:, kc:kc + 2, t0:t0 + N],
                                start=False, stop=(kc == KC - 2),
                                perf_mode=DR)
                    nc.scalar.activation(
                        ct[:, cq * 2:cq * 2 + 2, :N], ps[:, :, :N],
                        AF.Tanh, scale=1.0 / WS)

            def slab16(c0, n512=512):
                s = p16["fpool"].tile([128, KC, 512], F16, tag="slab")
                nc.sync.dma_start(
                    s[...], wh16_d[:, c0:c0 + 512]
                    .rearrange("(c p) m -> p c m", p=128))
                return s

            def gate16(garr, g, t0, N, pre_tiles):
                for half in range(2):
                    s = slab16(g * D + half * 512)
                    for cq in range(2):
                        ps = psum.tile([128, 2, 512], F32, tag="pp")
                        for j in range(2):
                            ch = half * 4 + cq * 2 + j
                            col = (cq * 2 + j) * 128
                            for kc in range(KC):
                                nc.tensor.matmul(
                                    ps[:, j, :N],
                                    s[:, kc, col:col + 128],
                                    Hf[:, kc, t0:t0 + N],
                                    start=(kc == 0), stop=False)
                            nc.tensor.matmul(
                                ps[:, j, :N], id1_sb[:, :],
                                pre_tiles[ch][:, :N],
                                start=False, stop=True)
                        ch0 = half * 4 + cq * 2
                        nc.scalar.activation(
                            garr[:, ch0:ch0 + 2, :N], ps[:, :, :N],
                            AF.Sigmoid)

            def psA16(zA, t0, N):
                for half in range(2):
                    s = slab16(3 * D + half * 512)
                    for cq in range(2):
                        psA = psum.tile([128, 2, 512], F32, tag="pp")
                        for j in range(2):
                            ch = half * 4 + cq * 2 + j
                            col = (cq * 2 + j) * 128
                            for kc in range(KC):
                                nc.tensor.matmul(
                                    psA[:, j, :N],
                                    s[:, kc, col:col + 128],
                                    Hf[:, kc, t0:t0 + N],
                                    start=(kc == 0), stop=(kc == KC - 1))
                        for j in range(2):
                            ch = half * 4 + cq * 2 + j
                            nc.scalar.activation(
                                zA[:, ch, :N], psA[:, j, :N],
                                AF.Identity, bias=bc_sb[:, ch:ch + 1])

            def ct0_16(ct, zA, t0, N, wcc):
                for cq in range(4):
                    ps = psum.tile([128, 2, 512], F32, tag="pp")
                    for j in range(2):
                        ch = cq * 2 + j
                        nc.tensor.matmul(
                            ps[:, j, :N], id1_sb[:, :],
                            zA[:, ch, :N], start=True, stop=False)
                        wcs = wcc[ch // 4]
                        ccol = (ch % 4) * 128
                        for kc in range(KC):
                            nc.tensor.matmul(
                                ps[:, j, :N],
                                wcs[:, kc, ccol:ccol + 128],
                                Cf[:, kc, t0:t0 + N],
                                start=False, stop=(kc == KC - 1))
                    for j in range(2):
                        ch = cq * 2 + j
                        nc.scalar.activation(
                            ct[:, ch, :N], ps[:, j, :N], AF.Tanh)

            def load_pre(g, t0, N):
                lst = []
                for gd in range(KC):
                    pt = prepool.tile([128, 512], F16, tag=f"p{g}", bufs=2)
                    nc.sync.dma_start(
                        pt[:, :N], pre_dram[g * KC + gd, :, t0:t0 + N])
                    lst.append(pt)
                return lst

            # ------------- job emitters -------------
            def emit_fi(jb):
                t0, N, mode = jb["t0"], jb["N"], jb["mode"]
                jb["f"] = gpool.tile([128, KC, N], F16, tag="fg", bufs=3, name=_nm("fg"))
                jb["i"] = gpool.tile([128, KC, N], F16, tag="ig", bufs=3, name=_nm("ig"))
                pf = load_pre(0, t0, N)
                pi = load_pre(1, t0, N)
                if mode == "8":
                    gate8(jb["f"], 0, t0, N, pf)
                    gate8(jb["i"], 1, t0, N, pi)
                else:
                    gate16(jb["f"], 0, t0, N, pf)
                    gate16(jb["i"], 1, t0, N, pi)

            def emit_opsA(jb):
                t0, N, mode = jb["t0"], jb["N"], jb["mode"]
                jb["o"] = gpool.tile([128, KC, N], F16, tag="og", bufs=2, name=_nm("og"))
                jb["zA"] = wk.tile([128, KC, N], F16, tag="zA", bufs=2, name=_nm("zA"))
                po = load_pre(2, t0, N)
                if mode == "8":
                    gate8(jb["o"], 2, t0, N, po)
                    psA8(jb["zA"], t0, N)
                else:
                    gate16(jb["o"], 2, t0, N, po)
                    psA16(jb["zA"], t0, N)

            def emit_ct0(jb):
                t0, N, mode = jb["t0"], jb["N"], jb["mode"]
                ct = wk.tile([128, KC, N], F16, tag="ct", bufs=2)
                jb["ct0"] = ct
                if mode == "8":
                    ct0_8(ct, jb["zA"], t0, N)
                else:
                    wcc = [p16["wccp"].tile([128, KC, 512], F16, tag="wcc",
                                            bufs=2, name=f"wcc{_k}")
                           for _k in range(2)]
                    for _k in range(2):
                        nc.sync.dma_start(
                            wcc[_k][...], wc16_d[:, _k * 512:_k * 512 + 512]
                            .rearrange("(c p) m -> p c m", p=128))
                    jb["wcc"] = wcc
                    ct0_16(ct, jb["zA"], t0, N, wcc)

            def emit_initial_ladder(jb):
                t0, N = jb["t0"], jb["N"]
                bb = wk.tile([128, KC, N], F16, tag="bb", bufs=1)
                for ch in range(KC):
                    nc.gpsimd.tensor_tensor(
                        bb[:, ch, :N], jb["i"][:, ch, :N],
                        jb["ct0"][:, ch, :N], ALU.mult)
                    do_scan(t0, N, jb["f"], bb, ch)
                jb["bb"] = bb

            def emit_refine(jb):
                t0, N, mode = jb["t0"], jb["N"], jb["mode"]
                ct = wk.tile([128, KC, N], F16, tag="ct", bufs=2)
                bb = wk.tile([128, KC, N], F16, tag="bb", bufs=1)
                scale = (1.0 / WS) if mode == "8" else 1.0
                for cq in range(4):
                    ps = psum.tile([128, 2, 512], F32, tag="pp")
                    for j in range(2):
                        ch = cq * 2 + j
                        nc.tensor.matmul(
                            ps[:, j, :N],
                            id16_sb[:, :] if mode == "8" else id1_sb[:, :],
                            jb["zA"][:, ch, :N], start=True, stop=False)
                        if mode == "8":
                            for kc in range(KC):
                                nc.tensor.matmul(
                                    ps[:, j, :N],
                                    wc8_sb[:, kc, ch * 128:(ch + 1) * 128],
                                    Cf[:, kc, t0:t0 + N],
                                    start=False, stop=(kc == KC - 1))
                        else:
                            wcs = jb["wcc"][ch // 4]
                            ccol = (ch % 4) * 128
                            for kc in range(KC):
                                nc.tensor.matmul(
                                    ps[:, j, :N],
                                    wcs[:, kc, ccol:ccol + 128],
                                    Cf[:, kc, t0:t0 + N],
                                    start=False, stop=(kc == KC - 1))
                        nc.scalar.activation(
                            ct[:, ch, :N], ps[:, j, :N], AF.Tanh, scale=scale)
                        nc.gpsimd.tensor_tensor(
                            bb[:, ch, :N], jb["i"][:, ch, :N],
                            ct[:, ch, :N], ALU.mult)
                        do_scan(t0, N, jb["f"], bb, ch)
                jb["bb"] = bb

            def emit_tail(jb):
                """Boundary flush only — keeps the next job's scans unblocked.
                The heavy do_h writes are deferred via jb["do_h"]."""
                t0, N = jb["t0"], jb["N"]
                w16, w8 = jb["write16"], jb["write8"]
                t_b = t0 + N
                # boundary flush (vector engine, reads live gate buffers)
                tb1 = tp.tile([128, KC, 1], F16, tag="tb1")
                nc.vector.tensor_tensor(tb1[...], jb["f"][:, :, N - 1:N],
                                        Cf[:, :, t_b - 1:t_b], ALU.mult)
                nc.vector.tensor_tensor(Cf[:, :, t_b:t_b + 1], tb1[...],
                                        jb["bb"][:, :, N - 1:N], ALU.add)
                if w8:
                    nc.gpsimd.tensor_copy(C8[:, :, t_b:t_b + 1],
                                          Cf[:, :, t_b:t_b + 1])
                tcb = tp.tile([128, KC, 1], F16, tag="tcb")
                nc.scalar.activation(tcb[...], Cf[:, :, t_b:t_b + 1], AF.Tanh)
                nc.vector.tensor_tensor(Hf[:, :, t_b:t_b + 1],
                                        jb["o"][:, :, N - 1:N],
                                        tcb[...], ALU.mult)
                if w8:
                    nc.gpsimd.tensor_copy(H8[:, :, t_b:t_b + 1],
                                          Hf[:, :, t_b:t_b + 1])

                def do_h():
                    if w8:
                        nc.vector.tensor_copy(C8[:, :, t0 + 1:t0 + N],
                                              Cf[:, :, t0 + 1:t0 + N])
                    tch = wk.tile([128, KC, N], F16, tag="ct", bufs=2,
                                  name=_nm("tch"))
                    nc.scalar.activation(tch[:, :, :N - 1],
                                         Cf[:, :, t0 + 1:t0 + N], AF.Tanh)
                    if w16:
                        nc.vector.tensor_tensor(Hf[:, :, t0 + 1:t0 + N],
                                                jb["o"][:, :, :N - 1],
                                                tch[:, :, :N - 1], ALU.mult)
                    if w8:
                        nc.gpsimd.tensor_tensor(H8[:, :, t0 + 1:t0 + N],
                                                jb["o"][:, :, :N - 1],
                                                tch[:, :, :N - 1], ALU.mult)
                jb["do_h"] = do_h

            # ---- job list for the pipelined main sweeps ----
            jobs = []
            for si, mode, ni in sweeps[1:]:
                for ti, (t0, N) in enumerate(M_TILES):
                    w16 = (mode == "f") or (si == last8_si) or (si == len(sweeps) - 1)
                    w8 = (mode == "8") and (si != last8_si)
                    jobs.append({"si": si, "mode": mode, "ni": ni, "ti": ti,
                                 "t0": t0, "N": N,
                                 "write16": w16, "write8": w8})
            K = len(jobs)

            def open_f_pools():
                if "fpool" not in p16:
                    wstack.close()   # free fp8 weight residency
                    p16["fpool"] = fstack.enter_context(
                        tc.tile_pool(name="fstream", bufs=2))
                    p16["wccp"] = fstack.enter_context(
                        tc.tile_pool(name="wccp", bufs=1))

            def boundary(k):
                # True if job k is fp16 but the previous job is fp8/absent
                return (k < K and jobs[k]["mode"] == "f"
                        and (k == 0 or jobs[k - 1]["mode"] == "8"))

            def prime(k):
                if boundary(k):
                    open_f_pools()
                emit_fi(jobs[k])
                emit_opsA(jobs[k])
                emit_ct0(jobs[k])
                if k + 1 < K and not boundary(k + 1):
                    emit_fi(jobs[k + 1])

            # ================= phase 1 + fused zero sweep =================
            pending_doh = [None]
            ni_z = sweeps[0][2]
            zjobs = [{"t0": t0, "N": N, "mode": "z",
                      "write16": False, "write8": True}
                     for (t0, N) in M_TILES]
            wc8_pend = [True]
            with tc.tile_pool(name="ph1", bufs=2) as ph1:
                for ti, (t0, N) in enumerate(M_TILES):
                    jb = zjobs[ti]
                    xT = ph1.tile([128, KC, 512], F16, tag="xT")
                    nc.sync.dma_start(
                        xT[:, :, :N],
                        x_d[:, t0:t0 + N].rearrange("(c p) t -> p c t", p=128))
                    jb["f"] = gpool.tile([128, KC, N], F16, tag="fg", bufs=3, name=_nm("fg"))
                    jb["i"] = gpool.tile([128, KC, N], F16, tag="ig", bufs=3, name=_nm("ig"))
                    jb["o"] = gpool.tile([128, KC, N], F16, tag="og", bufs=2, name=_nm("og"))
                    for g, garr in enumerate((jb["f"], jb["i"], jb["o"])):
                        for half in range(2):
                            wxs = ph1.tile([128, KC, 512], F16, tag="wxs")
                            nc.sync.dma_start(
                                wxs[...],
                                wx_d[:, g * D + half * 512: g * D + (half + 1) * 512]
                                .rearrange("(c p) m -> p c m", p=128))
                            if wc8_pend[0]:
                                wc8_pend[0] = False
                                nc.sync.dma_start(
                                    wc8_sb[...],
                                    wc8_d[:, :].rearrange("(c p) m -> p c m", p=128))
                            for gq in range(2):
                                ps = psum.tile([128, 2, 512], F32, tag="pp")
                                for j in range(2):
                                    gd = half * 4 + gq * 2 + j
                                    col = (gq * 2 + j) * 128
                                    for kc in range(KC):
                                        nc.tensor.matmul(
                                            ps[:, j, :N],
                                            wxs[:, kc, col:col + 128],
                                            xT[:, kc, :N],
                                            start=(kc == 0), stop=(kc == KC - 1))
                                for j in range(2):
                                    gd = half * 4 + gq * 2 + j
                                    pre_t = ph1.tile([128, 512], F16, tag="pre_t")
                                    nc.scalar.activation(
                                        pre_t[:, :N], ps[:, j, :N], AF.Identity,
                                        bias=bfio_sb[:, g, gd:gd + 1])
                                    nc.sync.dma_start(
                                        pre_dram[g * KC + gd, :, t0:t0 + N],
                                        pre_t[:, :N])
                                    nc.scalar.activation(
                                        garr[:, gd, :N], ps[:, j, :N], AF.Sigmoid,
                                        bias=bfio_sb[:, g, gd:gd + 1])
                    # ---- zero-sweep ladder for this m-tile ----
                    bb = wk.tile([128, KC, N], F16, tag="bb", bufs=1)
                    for ch in range(KC):
                        nc.vector.tensor_scalar(
                            bb[:, ch, :N], jb["i"][:, ch, :N],
                            tbc_sb[:, ch:ch + 1], None, ALU.mult)
                        do_scan(t0, N, jb["f"], bb, ch)
                    jb["bb"] = bb
                    for r in range(ni_z):
                        ct = wk.tile([128, KC, N], F16, tag="ct", bufs=2)
                        bb = wk.tile([128, KC, N], F16, tag="bb", bufs=1)
                        for cq in range(4):
                            ps = psum.tile([128, 2, 512], F32, tag="pp")
                            for j in range(2):
                                ch = cq * 2 + j
                                for kc in range(KC):
                                    nc.tensor.matmul(
                                        ps[:, j, :N],
                                        wc8_sb[:, kc, ch * 128:(ch + 1) * 128],
                                        Cf[:, kc, t0:t0 + N],
                                        start=(kc == 0), stop=(kc == KC - 1))
                                nc.scalar.activation(
                                    ct[:, ch, :N], ps[:, j, :N], AF.Tanh,
                                    bias=bc_sb[:, ch:ch + 1], scale=1.0 / WS)
                                nc.gpsimd.tensor_tensor(
                                    bb[:, ch, :N], jb["i"][:, ch, :N],
                                    ct[:, ch, :N], ALU.mult)
                                do_scan(t0, N, jb["f"], bb, ch)
                        jb["bb"] = bb
                    emit_tail(jb)
                    if ti < len(M_TILES) - 1:
                        jb["do_h"]()
                    else:
                        pending_doh[0] = jb["do_h"]

            # ================= pipelined main sweeps =================
            if K > 0 and jobs[0]["mode"] == "8":
                wpool = wstack.enter_context(tc.tile_pool(name="weights", bufs=1))
                wh8_ref["sb"] = wpool.tile([128, KC, 4 * D], F8, tag="wh8", name="wh8sb")
                nc.sync.dma_start(
                    wh8_ref["sb"][...],
                    wh8_d[:, :].rearrange("(c p) m -> p c m", p=128))
            if K > 0:
                prime(0)
            k = 0
            while k < K:
                jb = jobs[k]
                emit_initial_ladder(jb)
                if pending_doh[0] is not None:
                    pending_doh[0]()
                    pending_doh[0] = None
                nxt = k + 1
                # position-aware fillers: big PE work into each refine's
                # scan-ladder window. fi(k+2) doubles as a filler when the
                # next job is the small T2 tile.
                fillers = []
                fi_done = False
                if nxt < K and not boundary(nxt):
                    if jb["ti"] == 1:
                        fi_done = True
                        fillers = [
                            (lambda: emit_fi(jobs[k + 2]))
                            if (k + 2 < K and not boundary(k + 2))
                            else (lambda: None),
                            lambda: (emit_opsA(jobs[nxt]), emit_ct0(jobs[nxt])),
                        ]
                    else:
                        fillers = [
                            lambda: emit_opsA(jobs[nxt]),
                            lambda: (emit_ct0(jobs[nxt]),
                                     (emit_fi(jobs[k + 2])
                                      if (k + 2 < K and not boundary(k + 2))
                                      else None)),
                        ]
                        fi_done = True
                for r in range(jb["ni"]):
                    emit_refine(jb)
                    if fillers:
                        fillers.pop(0)()
                for fn in fillers:
                    fn()
                emit_tail(jb)
                pending_doh[0] = jb["do_h"]
                if nxt < K and boundary(nxt):
                    # pipeline restart across the 8->f boundary
                    prime(nxt)
                elif k + 2 < K and not fi_done and not boundary(k + 2):
                    emit_fi(jobs[k + 2])
                k += 1

            if pending_doh[0] is not None:
                pending_doh[0]()
                pending_doh[0] = None
            nc.sync.dma_start(h_out[...], Hf[:, :, :L + 1])
            wstack.close()
            fstack.close()

    nc.compile()
    return nc


# ------------------------- host side -------------------------

def _q8(a):
    return (np.asarray(a, np.float32) * WS).astype(ml_dtypes.float8_e4m3)


def _prep_core_inputs(inputs):
    x = np.asarray(inputs["target_seq"], np.float32)
    W_f = np.asarray(inputs["W_f"], np.float32)
    W_i = np.asarray(inputs["W_i"], np.float32)
    W_C = np.asarray(inputs["W_C"], np.float32)
    W_o = np.asarray(inputs["W_o"], np.float32)

    wh16 = np.concatenate(
        [W_f[:, :D].T, W_i[:, :D].T, W_o[:, :D].T, W_C[:, :D].T], axis=1
    ).astype(np.float16)                      # [D, 4D] cols = [f|i|o|C]
    wc16 = np.ascontiguousarray(W_C[:, D:].T).astype(np.float16)
    wh8 = _q8(wh16)
    wc8 = _q8(wc16)
    wx = np.concatenate(
        [W_f[:, D:].T, W_i[:, D:].T, W_o[:, D:].T], axis=1
    ).astype(np.float16)                      # [D, 3D]

    def vec_pc(v):
        return np.ascontiguousarray(np.asarray(v, np.float32).reshape(KC, 128).T)

    bias_fio = np.stack([vec_pc(inputs["b_f"]), vec_pc(inputs["b_i"]),
                         vec_pc(inputs["b_o"])], axis=1)  # [128, 3, 8]
    bc = vec_pc(inputs["b_C"])
    tanh_bc = np.tanh(bc).astype(np.float32)
    id16 = (np.eye(128) * WS).astype(np.float16)
    id1 = np.eye(128, dtype=np.float16)

    h0 = np.asarray(inputs["encoder_h"], np.float32)
    c0 = np.asarray(inputs["encoder_c"], np.float32)

    in_maps = []
    for core in range(N_CORES):
        if core == 0:
            rows = slice(0, L)
            hb = vec_pc(h0)[:, :, None]
            cb = vec_pc(c0)[:, :, None]
        else:
            rows = slice(1024 * core - DELTA, 1024 * core + 1024)
            hb = np.zeros((128, KC, 1), np.float32)
            cb = np.zeros((128, KC, 1), np.float32)
        x_chunk = np.ascontiguousarray(x[rows].T).astype(np.float16)
        in_maps.append({
            "wh8": wh8, "wc8": wc8, "wh16": wh16, "wc16": wc16, "wx": wx,
            "x": x_chunk,
            "bias_fio": bias_fio.astype(np.float32), "bc": bc.astype(np.float32),
            "tanh_bc": tanh_bc,
            "id16": id16, "id1": id1,
            "hb16": hb.astype(np.float16), "cb16": cb.astype(np.float16),
            "hb8": hb.astype(ml_dtypes.float8_e4m3),
            "cb8": cb.astype(ml_dtypes.float8_e4m3),
        })
    return in_maps


def _gather_output(results):
    out = np.empty((T, D), np.float32)
    for core in range(N_CORES):
        h = np.asarray(results[core]["h_out"]).reshape(128, KC, L + 1)
        chunk = np.transpose(h, (2, 1, 0)).reshape(L + 1, D).astype(np.float32)
        if core == 0:
            out[0:1024] = chunk[1:1025]
        else:
            out[1024 * core:1024 * (core + 1)] = chunk[DELTA + 1:L + 1]
    return out


_NC_CACHE = {}


def _get_nc(schedule=SCHEDULE, ni=NI):
    key = (schedule, ni)
    if key not in _NC_CACHE:
        _NC_CACHE[key] = build_nc(schedule, ni)
    return _NC_CACHE[key]


def kernel(**inputs) -> np.ndarray:
    nc = _get_nc()
    in_maps = _prep_core_inputs(inputs)
    res = run_bass_kernel_spmd(nc, in_maps, list(range(N_CORES)))
    return _gather_output(res.results)


if __name__ == "__main__":
    nc = build_nc()
    print("built ok")


# revision 17
# speedup vs baseline: 1.1182x; 1.1182x over previous
"""Trainium2 Bass kernel for the nn_Decoder LSTM problem — pipelined edition.

Same time-sharded Picard-sweep algorithm as the baseline (8 cores x 1072-step
chunks, fp8 DoubleRow bulk sweeps + fp16 polish, exact DVE scan for the linear
c recurrence), restructured for PE occupancy:

  - jobs = (sweep, tile) pairs; per-tile gate buffers (tags bufs=3/3/2) so the
    next job's gate/psA/ct0 matmuls are emitted into the current job's
    scan-ladder windows (software pipelining across tiles AND sweeps).
  - boundary flush runs directly on the vector engine reading the live gate
    buffers (no gpsimd copy chain, no deferral machinery).
  - recurrent-weight DMA queued after the first x/wx slabs so phase 1 starts
    immediately.
"""

import os
import sys
import numpy as np
import ml_dtypes

for _p in ("/opt/trn_rl_repo", "/root/.axon_site/_ro/trn_rl_repo"):
    if _p not in sys.path:
        sys.path.insert(0, _p)

import concourse.bass as bass
import concourse.bacc as bacc
import concourse.mybir as mybir
import concourse.tile as tile
from concourse.bass_utils import run_bass_kernel_spmd
from contextlib import ExitStack

D = 1024
T = 8192
KC = 8
DELTA = 48
L = 1024 + DELTA
N_CORES = 8
WS = 16.0          # fp8 weight scale
SCHEDULE = os.environ.get("LSTM_SCHED", "z88888ff")
NI = os.environ.get("LSTM_NI", "12")   # n_inner per sweep (last char repeats)

F16 = mybir.dt.float16
F32 = mybir.dt.float32
F8 = mybir.dt.float8e4
AF = mybir.ActivationFunctionType
ALU = mybir.AluOpType
DR = mybir.MatmulPerfMode.DoubleRow

M_TILES = [(0, 512), (512, 512), (1024, L - 1024)]


def build_nc(schedule=SCHEDULE, ni_str=NI):
    def ni_of(si):
        return int(ni_str[min(si, len(ni_str) - 1)])

    nc = bacc.Bacc(None, target_bir_lowering=False, debug=False)

    # ---- I/O ----
    wh8_d = nc.declare_dram_parameter("wh8", [D, 4 * D], F8, isOutput=False)
    wc8_d = nc.declare_dram_parameter("wc8", [D, D], F8, isOutput=False)
    wh16_d = nc.declare_dram_parameter("wh16", [D, 4 * D], F16, isOutput=False)
    wc16_d = nc.declare_dram_parameter("wc16", [D, D], F16, isOutput=False)
    wx_d = nc.declare_dram_parameter("wx", [D, 3 * D], F16, isOutput=False)
    x_d = nc.declare_dram_parameter("x", [D, L], F16, isOutput=False)
    bias_fio = nc.declare_dram_parameter("bias_fio", [128, 3, KC], F32, isOutput=False)
    bc_d = nc.declare_dram_parameter("bc", [128, KC], F32, isOutput=False)
    tanh_bc_d = nc.declare_dram_parameter("tanh_bc", [128, KC], F32, isOutput=False)
    id16_d = nc.declare_dram_parameter("id16", [128, 128], F16, isOutput=False)
    id1_d = nc.declare_dram_parameter("id1", [128, 128], F16, isOutput=False)
    hb16_d = nc.declare_dram_parameter("hb16", [128, KC, 1], F16, isOutput=False)
    cb16_d = nc.declare_dram_parameter("cb16", [128, KC, 1], F16, isOutput=False)
    hb8_d = nc.declare_dram_parameter("hb8", [128, KC, 1], F8, isOutput=False)
    cb8_d = nc.declare_dram_parameter("cb8", [128, KC, 1], F8, isOutput=False)
    h_out = nc.declare_dram_parameter("h_out", [128, KC, L + 1], F16, isOutput=True)

    pre_dram = nc.dram_tensor("pre_scratch", [3 * KC, 128, L], F16)

    # sweep descriptors: schedule[0] must be 'z' (fused with phase 1)
    assert schedule[0] == "z"
    sweeps = [(si, m, ni_of(si)) for si, m in enumerate(schedule)]
    last8_si = max((si for si, m, _ in sweeps if m == "8"), default=None)

    with tile.TileContext(nc) as tc:
        with (
            tc.tile_pool(name="const", bufs=1) as constp,
            tc.tile_pool(name="psum", bufs=4, space="PSUM") as psum,
            tc.tile_pool(name="state", bufs=1) as spool,
            tc.tile_pool(name="gates", bufs=1) as gpool,
            tc.tile_pool(name="work", bufs=1) as wk,
            tc.tile_pool(name="prestream", bufs=2) as prepool,
            tc.tile_pool(name="tiny", bufs=2) as tp,
        ):
            _nmc = [0]

            def _nm(p):
                _nmc[0] += 1
                return f"{p}{_nmc[0]}"

            wstack = ExitStack()
            fstack = ExitStack()
            p16 = {}   # filled with fpool/wcc pool at the 8->f boundary

            bfio_sb = constp.tile([128, 3, KC], F32, tag="bfio")
            bc_sb = constp.tile([128, KC], F32, tag="bc")
            tbc_sb = constp.tile([128, KC], F32, tag="tbc")
            id16_sb = constp.tile([128, 128], F16, tag="id16")
            id1_sb = constp.tile([128, 128], F16, tag="id1")
            nc.sync.dma_start(bfio_sb[...], bias_fio[...])
            nc.sync.dma_start(bc_sb[...], bc_d[...])
            nc.sync.dma_start(tbc_sb[...], tanh_bc_d[...])
            nc.sync.dma_start(id16_sb[...], id16_d[...])
            nc.sync.dma_start(id1_sb[...], id1_d[...])

            wc8_sb = constp.tile([128, KC, D], F8, tag="wc8")
            wh8_ref = {}   # wh8_ref["sb"] set when the wh8 pool opens

            LP = L + 4
            Hf = spool.tile([128, KC, LP], F16, tag="H")
            Cf = spool.tile([128, KC, LP], F16, tag="C")
            H8 = spool.tile([128, KC, LP], F8, tag="H8")
            C8 = spool.tile([128, KC, LP], F8, tag="C8")
            nc.sync.dma_start(Hf[:, :, 0:1], hb16_d[...])
            nc.sync.dma_start(Cf[:, :, 0:1], cb16_d[...])
            nc.sync.dma_start(H8[:, :, 0:1], hb8_d[...])
            nc.sync.dma_start(C8[:, :, 0:1], cb8_d[...])

            # ------------- shared helpers -------------
            def do_scan(t0, N, fgt, bb, ch):
                nc.vector.tensor_tensor_scan(
                    Cf[:, ch, t0 + 1:t0 + N],
                    fgt[:, ch, :N - 1],
                    bb[:, ch, :N - 1],
                    Cf[:, ch, t0:t0 + 1],
                    ALU.mult, ALU.add,
                )

            def gate8(garr, g, t0, N, pre_tiles):
                for cq in range(4):
                    ps = psum.tile([128, 2, 512], F32, tag="pp")
                    for j in range(2):
                        ch = cq * 2 + j
                        col = g * D + ch * 128
                        for kc in range(0, KC, 2):
                            nc.tensor.matmul(
                                ps[:, j, :N],
                                wh8_ref["sb"][:, kc:kc + 2, col:col + 128],
                                H8[:, kc:kc + 2, t0:t0 + N],
                                start=(kc == 0), stop=False, perf_mode=DR)
                        nc.tensor.matmul(
                            ps[:, j, :N], id16_sb[:, :],
                            pre_tiles[ch][:, :N], start=False, stop=True)
                    nc.scalar.activation(
                        garr[:, cq * 2:cq * 2 + 2, :N], ps[:, :, :N],
                        AF.Sigmoid, scale=1.0 / WS)

            def psA8(zA, t0, N):
                for cq in range(4):
                    psA = psum.tile([128, 2, 512], F32, tag="pp")
                    for j in range(2):
                        ch = cq * 2 + j
                        col = 3 * D + ch * 128
                        for kc in range(0, KC, 2):
                            nc.tensor.matmul(
                                psA[:, j, :N],
                                wh8_ref["sb"][:, kc:kc + 2, col:col + 128],
                                H8[:, kc:kc + 2, t0:t0 + N],
                                start=(kc == 0), stop=(kc == KC - 2),
                                perf_mode=DR)
                    for j in range(2):
                        ch = cq * 2 + j
                        nc.scalar.activation(
                            zA[:, ch, :N], psA[:, j, :N], AF.Identity,
                            bias=bc_sb[:, ch:ch + 1], scale=1.0 / WS)

            def ct0_8(ct, zA, t0, N):
                for cq in range(4):
                    ps = psum.tile([128, 2, 512], F32, tag="pp")
                    for j in range(2):
                        ch = cq * 2 + j
                        nc.tensor.matmul(
                            ps[:, j, :N], id16_sb[:, :], zA[:, ch, :N],
                            start=True, stop=False)
                        for kc in range(0, KC, 2):
                            nc.tensor.matmul(
                                ps[:, j, :N],
                                wc8_sb[:, kc:kc + 2, ch * 128:(ch + 1) * 128],
                                C8[:, kc:kc + 2, t0:t0 + N],
                                start=False, stop=(kc == KC - 2),
                                perf_mode=DR)
                    nc.scalar.activation(
                        ct[:, cq * 2:cq * 2 + 2, :N], ps[:, :, :N],
                        AF.Tanh, scale=1.0 / WS)

            def slab16(c0, n512=512):
                s = p16["fpool"].tile([128, KC, 512], F16, tag="slab")
                nc.sync.dma_start(
                    s[...], wh16_d[:, c0:c0 + 512]
                    .rearrange("(c p) m -> p c m", p=128))
                return s

            def gate16(garr, g, t0, N, pre_tiles):
                for half in range(2):
                    s = slab16(g * D + half * 512)
                    for cq in range(2):
                        ps = psum.tile([128, 2, 512], F32, tag="pp")
                        for j in range(2):
                            ch = half * 4 + cq * 2 + j
                            col = (cq * 2 + j) * 128
                            for kc in range(KC):
                                nc.tensor.matmul(
                                    ps[:, j, :N],
                                    s[:, kc, col:col + 128],
                                    Hf[:, kc, t0:t0 + N],
                                    start=(kc == 0), stop=False)
                            nc.tensor.matmul(
                                ps[:, j, :N], id1_sb[:, :],
                                pre_tiles[ch][:, :N],
                                start=False, stop=True)
                        ch0 = half * 4 + cq * 2
                        nc.scalar.activation(
                            garr[:, ch0:ch0 + 2, :N], ps[:, :, :N],
                            AF.Sigmoid)

            def psA16(zA, t0, N):
                for half in range(2):
                    s = slab16(3 * D + half * 512)
                    for cq in range(2):
                        psA = psum.tile([128, 2, 512], F32, tag="pp")
                        for j in range(2):
                            ch = half * 4 + cq * 2 + j
                            col = (cq * 2 + j) * 128
                            for kc in range(KC):
                                nc.tensor.matmul(
                                    psA[:, j, :N],
                                    s[:, kc, col:col + 128],
                                    Hf[:, kc, t0:t0 + N],
                                    start=(kc == 0), stop=(kc == KC - 1))
                        for j in range(2):
                            ch = half * 4 + cq * 2 + j
                            nc.scalar.activation(
                                zA[:, ch, :N], psA[:, j, :N],
                                AF.Identity, bias=bc_sb[:, ch:ch + 1])

            def ct0_16(ct, zA, t0, N, wcc):
                for cq in range(4):
                    ps = psum.tile([128, 2, 512], F32, tag="pp")
                    for j in range(2):
                        ch = cq * 2 + j
                        nc.tensor.matmul(
                            ps[:, j, :N], id1_sb[:, :],
                            zA[:, ch, :N], start=True, stop=False)
                        wcs = wcc[ch // 4]
                        ccol = (ch % 4) * 128
                        for kc in range(KC):
                            nc.tensor.matmul(
                                ps[:, j, :N],
                                wcs[:, kc, ccol:ccol + 128],
                                Cf[:, kc, t0:t0 + N],
                                start=False, stop=(kc == KC - 1))
                    for j in range(2):
                        ch = cq * 2 + j
                        nc.scalar.activation(
                            ct[:, ch, :N], ps[:, j, :N], AF.Tanh)

            def load_pre(g, t0, N):
                lst = []
                for gd in range(KC):
                    pt = prepool.tile([128, 512], F16, tag=f"p{g}", bufs=2)
                    nc.sync.dma_start(
                        pt[:, :N], pre_dram[g * KC + gd, :, t0:t0 + N])
                    lst.append(pt)
                return lst

            # ------------- job emitters -------------
            def emit_fi(jb):
                t0, N, mode = jb["t0"], jb["N"], jb["mode"]
                jb["f"] = gpool.tile([128, KC, N], F16, tag="fg", bufs=3, name=_nm("fg"))
                jb["i"] = gpool.tile([128, KC, N], F16, tag="ig", bufs=3, name=_nm("ig"))
                pf = load_pre(0, t0, N)
                pi = load_pre(1, t0, N)
                if mode == "8":
                    gate8(jb["f"], 0, t0, N, pf)
                    gate8(jb["i"], 1, t0, N, pi)
                else:
                    gate16(jb["f"], 0, t0, N, pf)
                    gate16(jb["i"], 1, t0, N, pi)

            def emit_opsA(jb):
                t0, N, mode = jb["t0"], jb["N"], jb["mode"]
                jb["o"] = gpool.tile([128, KC, N], F16, tag="og", bufs=2, name=_nm("og"))
                jb["zA"] = wk.tile([128, KC, N], F16, tag="zA", bufs=2, name=_nm("zA"))
                po = load_pre(2, t0, N)
                if mode == "8":
                    gate8(jb["o"], 2, t0, N, po)
                    psA8(jb["zA"], t0, N)
                else:
                    gate16(jb["o"], 2, t0, N, po)
                    psA16(jb["zA"], t0, N)

            def emit_ct0(jb):
                t0, N, mode = jb["t0"], jb["N"], jb["mode"]
                ct = wk.tile([128, KC, N], F16, tag="ct", bufs=2)
                jb["ct0"] = ct
                if mode == "8":
                    ct0_8(ct, jb["zA"], t0, N)
                else:
                    wcc = [p16["wccp"].tile([128, KC, 512], F16, tag="wcc",
                                            bufs=2, name=f"wcc{_k}")
                           for _k in range(2)]
                    for _k in range(2):
                        nc.sync.dma_start(
                            wcc[_k][...], wc16_d[:, _k * 512:_k * 512 + 512]
                            .rearrange("(c p) m -> p c m", p=128))
                    jb["wcc"] = wcc
                    ct0_16(ct, jb["zA"], t0, N, wcc)

            def emit_initial_ladder(jb):
                t0, N = jb["t0"], jb["N"]
                bb = wk.tile([128, KC, N], F16, tag="bb", bufs=1)
                for ch in range(KC):
                    nc.vector.tensor_tensor(
                        bb[:, ch, :N], jb["i"][:, ch, :N],
                        jb["ct0"][:, ch, :N], ALU.mult)
                    do_scan(t0, N, jb["f"], bb, ch)
                jb["bb"] = bb

            def emit_refine(jb):
                t0, N, mode = jb["t0"], jb["N"], jb["mode"]
                ct = wk.tile([128, KC, N], F16, tag="ct", bufs=2)
                bb = wk.tile([128, KC, N], F16, tag="bb", bufs=1)
                scale = (1.0 / WS) if mode == "8" else 1.0
                for cq in range(4):
                    ps = psum.tile([128, 2, 512], F32, tag="pp")
                    for j in range(2):
                        ch = cq * 2 + j
                        nc.tensor.matmul(
                            ps[:, j, :N],
                            id16_sb[:, :] if mode == "8" else id1_sb[:, :],
                            jb["zA"][:, ch, :N], start=True, stop=False)
                        if mode == "8":
                            for kc in range(KC):
                                nc.tensor.matmul(
                                    ps[:, j, :N],
                                    wc8_sb[:, kc, ch * 128:(ch + 1) * 128],
                                    Cf[:, kc, t0:t0 + N],
                                    start=False, stop=(kc == KC - 1))
                        else:
                            wcs = jb["wcc"][ch // 4]
                            ccol = (ch % 4) * 128
                            for kc in range(KC):
                                nc.tensor.matmul(
                                    ps[:, j, :N],
                                    wcs[:, kc, ccol:ccol + 128],
                                    Cf[:, kc, t0:t0 + N],
                                    start=False, stop=(kc == KC - 1))
                        nc.scalar.activation(
                            ct[:, ch, :N], ps[:, j, :N], AF.Tanh, scale=scale)
                        nc.vector.tensor_tensor(
                            bb[:, ch, :N], jb["i"][:, ch, :N],
                            ct[:, ch, :N], ALU.mult)
                        do_scan(t0, N, jb["f"], bb, ch)
                jb["bb"] = bb

            def emit_tail(jb):
                """Boundary flush only — keeps the next job's scans unblocked.
                The heavy do_h writes are deferred via jb["do_h"]."""
                t0, N = jb["t0"], jb["N"]
                w16, w8 = jb["write16"], jb["write8"]
                t_b = t0 + N
                # boundary flush (vector engine, reads live gate buffers)
                tb1 = tp.tile([128, KC, 1], F16, tag="tb1")
                nc.vector.tensor_tensor(tb1[...], jb["f"][:, :, N - 1:N],
                                        Cf[:, :, t_b - 1:t_b], ALU.mult)
                nc.vector.tensor_tensor(Cf[:, :, t_b:t_b + 1], tb1[...],
                                        jb["bb"][:, :, N - 1:N], ALU.add)
                if w8:
                    nc.vector.tensor_copy(C8[:, :, t_b:t_b + 1],
                                          Cf[:, :, t_b:t_b + 1])
                tcb = tp.tile([128, KC, 1], F16, tag="tcb")
                nc.scalar.activation(tcb[...], Cf[:, :, t_b:t_b + 1], AF.Tanh)
                nc.vector.tensor_tensor(Hf[:, :, t_b:t_b + 1],
                                        jb["o"][:, :, N - 1:N],
                                        tcb[...], ALU.mult)
                if w8:
                    nc.vector.tensor_copy(H8[:, :, t_b:t_b + 1],
                                          Hf[:, :, t_b:t_b + 1])

                def do_h():
                    if w8:
                        nc.vector.tensor_copy(C8[:, :, t0 + 1:t0 + N],
                                              Cf[:, :, t0 + 1:t0 + N])
                    tch = wk.tile([128, KC, N], F16, tag="ct", bufs=2,
                                  name=_nm("tch"))
                    nc.scalar.activation(tch[:, :, :N - 1],
                                         Cf[:, :, t0 + 1:t0 + N], AF.Tanh)
                    if w16:
                        nc.vector.tensor_tensor(Hf[:, :, t0 + 1:t0 + N],
                                                jb["o"][:, :, :N - 1],
                                                tch[:, :, :N - 1], ALU.mult)
                    if w8:
                        nc.vector.tensor_tensor(H8[:, :, t0 + 1:t0 + N],
                                                jb["o"][:, :, :N - 1],
                                                tch[:, :, :N - 1], ALU.mult)
                jb["do_h"] = do_h

            # ---- job list for the pipelined main sweeps ----
            jobs = []
            for si, mode, ni in sweeps[1:]:
                for ti, (t0, N) in enumerate(M_TILES):
                    w16 = (mode == "f") or (si == last8_si) or (si == len(sweeps) - 1)
                    w8 = (mode == "8") and (si != last8_si)
                    jobs.append({"si": si, "mode": mode, "ni": ni, "ti": ti,
                                 "t0": t0, "N": N,
                                 "write16": w16, "write8": w8})
            K = len(jobs)

            def open_f_pools():
                if "fpool" not in p16:
                    wstack.close()   # free fp8 weight residency
                    p16["fpool"] = fstack.enter_context(
                        tc.tile_pool(name="fstream", bufs=2))
                    p16["wccp"] = fstack.enter_context(
                        tc.tile_pool(name="wccp", bufs=1))

            def boundary(k):
                # True if job k is fp16 but the previous job is fp8/absent
                return (k < K and jobs[k]["mode"] == "f"
                        and (k == 0 or jobs[k - 1]["mode"] == "8"))

            def prime(k):
                if boundary(k):
                    open_f_pools()
                emit_fi(jobs[k])
                emit_opsA(jobs[k])
                emit_ct0(jobs[k])
                if k + 1 < K and not boundary(k + 1):
                    emit_fi(jobs[k + 1])

            # ================= phase 1 + fused zero sweep =================
            pending_doh = [None]
            ni_z = sweeps[0][2]
            zjobs = [{"t0": t0, "N": N, "mode": "z",
                      "write16": False, "write8": True}
                     for (t0, N) in M_TILES]
            wc8_pend = [True]
            with tc.tile_pool(name="ph1", bufs=2) as ph1:
                for ti, (t0, N) in enumerate(M_TILES):
                    jb = zjobs[ti]
                    xT = ph1.tile([128, KC, 512], F16, tag="xT")
                    nc.sync.dma_start(
                        xT[:, :, :N],
                        x_d[:, t0:t0 + N].rearrange("(c p) t -> p c t", p=128))
                    jb["f"] = gpool.tile([128, KC, N], F16, tag="fg", bufs=3, name=_nm("fg"))
                    jb["i"] = gpool.tile([128, KC, N], F16, tag="ig", bufs=3, name=_nm("ig"))
                    jb["o"] = gpool.tile([128, KC, N], F16, tag="og", bufs=2, name=_nm("og"))
                    for g, garr in enumerate((jb["f"], jb["i"], jb["o"])):
                        for half in range(2):
                            wxs = ph1.tile([128, KC, 512], F16, tag="wxs")
                            nc.sync.dma_start(
                                wxs[...],
                                wx_d[:, g * D + half * 512: g * D + (half + 1) * 512]
                                .rearrange("(c p) m -> p c m", p=128))
                            if wc8_pend[0]:
                                wc8_pend[0] = False
                                nc.sync.dma_start(
                                    wc8_sb[...],
                                    wc8_d[:, :].rearrange("(c p) m -> p c m", p=128))
                            for gq in range(2):
                                ps = psum.tile([128, 2, 512], F32, tag="pp")
                                for j in range(2):
                                    gd = half * 4 + gq * 2 + j
                                    col = (gq * 2 + j) * 128
                                    for kc in range(KC):
                                        nc.tensor.matmul(
                                            ps[:, j, :N],
                                            wxs[:, kc, col:col + 128],
                                            xT[:, kc, :N],
                                            start=(kc == 0), stop=(kc == KC - 1))
                                for j in range(2):
                                    gd = half * 4 + gq * 2 + j
                                    pre_t = ph1.tile([128, 512], F16, tag="pre_t")
                                    nc.scalar.activation(
                                        pre_t[:, :N], ps[:, j, :N], AF.Identity,
                                        bias=bfio_sb[:, g, gd:gd + 1])
                                    nc.sync.dma_start(
                                        pre_dram[g * KC + gd, :, t0:t0 + N],
                                        pre_t[:, :N])
                                    nc.scalar.activation(
                                        garr[:, gd, :N], ps[:, j, :N], AF.Sigmoid,
                                        bias=bfio_sb[:, g, gd:gd + 1])
                    # ---- zero-sweep ladder for this m-tile ----
                    bb = wk.tile([128, KC, N], F16, tag="bb", bufs=1)
                    for ch in range(KC):
                        nc.vector.tensor_scalar(
                            bb[:, ch, :N], jb["i"][:, ch, :N],
                            tbc_sb[:, ch:ch + 1], None, ALU.mult)
                        do_scan(t0, N, jb["f"], bb, ch)
                    jb["bb"] = bb
                    for r in range(ni_z):
                        ct = wk.tile([128, KC, N], F16, tag="ct", bufs=2)
                        bb = wk.tile([128, KC, N], F16, tag="bb", bufs=1)
                        for cq in range(4):
                            ps = psum.tile([128, 2, 512], F32, tag="pp")
                            for j in range(2):
                                ch = cq * 2 + j
                                for kc in range(KC):
                                    nc.tensor.matmul(
                                        ps[:, j, :N],
                                        wc8_sb[:, kc, ch * 128:(ch + 1) * 128],
                                        Cf[:, kc, t0:t0 + N],
                                        start=(kc == 0), stop=(kc == KC - 1))
                                nc.scalar.activation(
                                    ct[:, ch, :N], ps[:, j, :N], AF.Tanh,
                                    bias=bc_sb[:, ch:ch + 1], scale=1.0 / WS)
                                nc.vector.tensor_tensor(
                                    bb[:, ch, :N], jb["i"][:, ch, :N],
                                    ct[:, ch, :N], ALU.mult)
                                do_scan(t0, N, jb["f"], bb, ch)
                        jb["bb"] = bb
                    emit_tail(jb)
                    if ti < len(M_TILES) - 1:
                        jb["do_h"]()
                    else:
                        pending_doh[0] = jb["do_h"]

            # ================= pipelined main sweeps =================
            if K > 0 and jobs[0]["mode"] == "8":
                wpool = wstack.enter_context(tc.tile_pool(name="weights", bufs=1))
                wh8_ref["sb"] = wpool.tile([128, KC, 4 * D], F8, tag="wh8", name="wh8sb")
                nc.sync.dma_start(
                    wh8_ref["sb"][...],
                    wh8_d[:, :].rearrange("(c p) m -> p c m", p=128))
            if K > 0:
                prime(0)
            k = 0
            while k < K:
                jb = jobs[k]
                emit_initial_ladder(jb)
                if pending_doh[0] is not None:
                    pending_doh[0]()
                    pending_doh[0] = None
                nxt = k + 1
                # position-aware fillers: big PE work into each refine's
                # scan-ladder window. fi(k+2) doubles as a filler when the
                # next job is the small T2 tile.
                fillers = []
                fi_done = False
                if nxt < K and not boundary(nxt):
                    if jb["ti"] == 1:
                        fi_done = True
                        fillers = [
                            (lambda: emit_fi(jobs[k + 2]))
                            if (k + 2 < K and not boundary(k + 2))
                            else (lambda: None),
                            lambda: (emit_opsA(jobs[nxt]), emit_ct0(jobs[nxt])),
                        ]
                    else:
                        fillers = [
                            lambda: emit_opsA(jobs[nxt]),
                            lambda: (emit_ct0(jobs[nxt]),
                                     (emit_fi(jobs[k + 2])
                                      if (k + 2 < K and not boundary(k + 2))
                                      else None)),
                        ]
                        fi_done = True
                for r in range(jb["ni"]):
                    emit_refine(jb)
                    if fillers:
                        fillers.pop(0)()
                for fn in fillers:
                    fn()
                emit_tail(jb)
                pending_doh[0] = jb["do_h"]
                if nxt < K and boundary(nxt):
                    # pipeline restart across the 8->f boundary
                    prime(nxt)
                elif k + 2 < K and not fi_done and not boundary(k + 2):
                    emit_fi(jobs[k + 2])
                k += 1

            if pending_doh[0] is not None:
                pending_doh[0]()
                pending_doh[0] = None
            nc.sync.dma_start(h_out[...], Hf[:, :, :L + 1])
            wstack.close()
            fstack.close()

    nc.compile()
    return nc


# ------------------------- host side -------------------------

def _q8(a):
    return (np.asarray(a, np.float32) * WS).astype(ml_dtypes.float8_e4m3)


def _prep_core_inputs(inputs):
    x = np.asarray(inputs["target_seq"], np.float32)
    W_f = np.asarray(inputs["W_f"], np.float32)
    W_i = np.asarray(inputs["W_i"], np.float32)
    W_C = np.asarray(inputs["W_C"], np.float32)
    W_o = np.asarray(inputs["W_o"], np.float32)

    wh16 = np.concatenate(
        [W_f[:, :D].T, W_i[:, :D].T, W_o[:, :D].T, W_C[:, :D].T], axis=1
    ).astype(np.float16)                      # [D, 4D] cols = [f|i|o|C]
    wc16 = np.ascontiguousarray(W_C[:, D:].T).astype(np.float16)
    wh8 = _q8(wh16)
    wc8 = _q8(wc16)
    wx = np.concatenate(
        [W_f[:, D:].T, W_i[:, D:].T, W_o[:, D:].T], axis=1
    ).astype(np.float16)                      # [D, 3D]

    def vec_pc(v):
        return np.ascontiguousarray(np.asarray(v, np.float32).reshape(KC, 128).T)

    bias_fio = np.stack([vec_pc(inputs["b_f"]), vec_pc(inputs["b_i"]),
                         vec_pc(inputs["b_o"])], axis=1)  # [128, 3, 8]
    bc = vec_pc(inputs["b_C"])
    tanh_bc = np.tanh(bc).astype(np.float32)
    id16 = (np.eye(128) * WS).astype(np.float16)
    id1 = np.eye(128, dtype=np.float16)

    h0 = np.asarray(inputs["encoder_h"], np.float32)
    c0 = np.asarray(inputs["encoder_c"], np.float32)

    in_maps = []
    for core in range(N_CORES):
        if core == 0:
            rows = slice(0, L)
            hb = vec_pc(h0)[:, :, None]
            cb = vec_pc(c0)[:, :, None]
        else:
            rows = slice(1024 * core - DELTA, 1024 * core + 1024)
            hb = np.zeros((128, KC, 1), np.float32)
            cb = np.zeros((128, KC, 1), np.float32)
        x_chunk = np.ascontiguousarray(x[rows].T).astype(np.float16)
        in_maps.append({
            "wh8": wh8, "wc8": wc8, "wh16": wh16, "wc16": wc16, "wx": wx,
            "x": x_chunk,
            "bias_fio": bias_fio.astype(np.float32), "bc": bc.astype(np.float32),
            "tanh_bc": tanh_bc,
            "id16": id16, "id1": id1,
            "hb16": hb.astype(np.float16), "cb16": cb.astype(np.float16),
            "hb8": hb.astype(ml_dtypes.float8_e4m3),
            "cb8": cb.astype(ml_dtypes.float8_e4m3),
        })
    return in_maps


def _gather_output(results):
    out = np.empty((T, D), np.float32)
    for core in range(N_CORES):
        h = np.asarray(results[core]["h_out"]).reshape(128, KC, L + 1)
        chunk = np.transpose(h, (2, 1, 0)).reshape(L + 1, D).astype(np.float32)
        if core == 0:
            out[0:1024] = chunk[1:1025]
        else:
            out[1024 * core:1024 * (core + 1)] = chunk[DELTA + 1:L + 1]
    return out


_NC_CACHE = {}


def _get_nc(schedule=SCHEDULE, ni=NI):
    key = (schedule, ni)
    if key not in _NC_CACHE:
        _NC_CACHE[key] = build_nc(schedule, ni)
    return _NC_CACHE[key]


def kernel(**inputs) -> np.ndarray:
    nc = _get_nc()
    in_maps = _prep_core_inputs(inputs)
    res = run_bass_kernel_spmd(nc, in_maps, list(range(N_CORES)))
    return _gather_output(res.results)


if __name__ == "__main__":
    nc = build_nc()
    print("built ok")
